# revision 2
# baseline (speedup 1.0000x reference)
"""MultiHeadAttention Trainium2 kernel (8 NeuronCores, SPMD).

Problem: B=2, L=2048, DK=DV=512, H=8, dh=64.
  Q = q @ WQ[h]; K = k @ WK[h]; V = v @ WV[h]       (per head)
  y = Q K^T / sqrt(L); z = softmax(y, axis=QUERY); out = z @ V
  concat heads on feature dim.

Sharding: 16 (b,h) pairs over 8 cores -> 2 heads (same batch) per core.
Host marshals per-core inputs: activations transposed to [D, L] and cast to
bf16, the core's two heads' weights packed to [D, 128].

Device layout (per core, heads h0/h1 packed in partition halves):
  QT/KT = [e-pack(128), L]   (rows 0:64 head0's Q^T, 64:128 head1's)
  S'[k-tile, q] scores with k on partitions, q on free axis -> softmax over q
  is a free-axis row-sum: exp on ScalarE with fused accum_out row sums.
  1/D is folded into V rows (k-indexed), then out^T[ev,q] accumulates in PSUM.
Core output: [128, 2048] f32 = two heads' out^T stacked; host transposes back.
"""

import math

import numpy as np

B = 2
L = 2048
DK = 512
H = 8
DH = 64
P = 128
NKT = L // P  # 16 k-tiles
NQC = L // 512  # 4 q-chunks
NDC = DK // P  # 4 d-chunks
N_CORES = 8

_CACHE = {}


def _build_program():
    import concourse.bass as bass
    import concourse.tile as tile
    from concourse import bacc, mybir
    from concourse.bass import ts

    f32 = mybir.dt.float32
    bf16 = mybir.dt.bfloat16
    AF = mybir.ActivationFunctionType
    SCALE = 1.0 / math.sqrt(float(L))

    nc = bacc.Bacc("TRN2", target_bir_lowering=False, debug=False)

    qt_d = nc.dram_tensor("qt", [DK, L], bf16, kind="ExternalInput")
    kt_d = nc.dram_tensor("kt", [DK, L], bf16, kind="ExternalInput")
    vt_d = nc.dram_tensor("vt", [DK, L], bf16, kind="ExternalInput")
    wq_d = nc.dram_tensor("wq", [DK, P], bf16, kind="ExternalInput")
    wk_d = nc.dram_tensor("wk", [DK, P], bf16, kind="ExternalInput")
    wv_d = nc.dram_tensor("wv", [DK, P], bf16, kind="ExternalInput")
    out_d = nc.dram_tensor("out", [P, L], f32, kind="ExternalOutput")

    with tile.TileContext(nc) as tc:
        with (
            tc.tile_pool(name="consts", bufs=1) as consts,
            tc.tile_pool(name="xin", bufs=1) as xin,
            tc.tile_pool(name="proj", bufs=1) as proj,
            tc.tile_pool(name="epool", bufs=4) as epool,
            tc.tile_pool(name="vspool", bufs=4) as vspool,
            tc.tile_pool(name="stats", bufs=1) as stats,
            tc.tile_pool(name="outp", bufs=1) as outp,
            tc.tile_pool(name="spsum", bufs=2, space="PSUM") as spsum,
            tc.tile_pool(name="avpsum", bufs=1, space="PSUM") as avpsum,
        ):
            # ---- weights ----
            wq_s = consts.tile([P, NDC, P], bf16)
            nc.sync.dma_start(wq_s[:], wq_d.rearrange("(o p) e -> p o e", p=P))
            wk_s = consts.tile([P, NDC, P], bf16)
            nc.sync.dma_start(wk_s[:], wk_d.rearrange("(o p) e -> p o e", p=P))
            wv_s = consts.tile([P, NDC, P], bf16)
            nc.sync.dma_start(wv_s[:], wv_d.rearrange("(o p) e -> p o e", p=P))

            # ---- transposed activations [128, NDC, L] ----
            qt_s = xin.tile([P, NDC, L], bf16)
            nc.sync.dma_start(qt_s[:], qt_d.rearrange("(o p) l -> p o l", p=P))
            kt_s = xin.tile([P, NDC, L], bf16)
            nc.sync.dma_start(kt_s[:], kt_d.rearrange("(o p) l -> p o l", p=P))
            vt_s = xin.tile([P, NDC, L], bf16)
            nc.sync.dma_start(vt_s[:], vt_d.rearrange("(o p) l -> p o l", p=P))

            QT = proj.tile([P, L], bf16)
            KT = proj.tile([P, L], bf16)
            Vf = proj.tile([P, NKT, P], f32)

            Dsum = stats.tile([P, 2, NKT, 2], f32)
            Dtot = stats.tile([P, 2, NKT], f32)
            Drec = stats.tile([P, 2, NKT], f32)

            # AV accumulator: rows 0:64 head0 out^T, 64:128 head1 out^T
            ovs = avpsum.tile([P, L], f32)

            # ---- Q projection (full, needed before any scores) ----
            for qc in range(NQC):
                ps = spsum.tile([P, 1024], f32, tag="sco")
                for dc in range(NDC):
                    nc.tensor.matmul(
                        ps[:, 0:512],
                        lhsT=wq_s[:, dc, :],
                        rhs=qt_s[:, dc, ts(qc, 512)],
                        start=(dc == 0),
                        stop=(dc == NDC - 1),
                    )
                nc.vector.tensor_copy(QT[:, ts(qc, 512)], ps[:, 0:512])

            # ---- main loop over k-tiles ----
            for kt in range(NKT):
                # K projection, one 512-wide chunk every 4 k-tiles
                if kt % 4 == 0:
                    kc4 = kt // 4
                    ps = spsum.tile([P, 1024], f32, tag="sco")
                    for dc in range(NDC):
                        nc.tensor.matmul(
                            ps[:, 0:512],
                            lhsT=wk_s[:, dc, :],
                            rhs=kt_s[:, dc, ts(kc4, 512)],
                            start=(dc == 0),
                            stop=(dc == NDC - 1),
                        )
                    nc.vector.tensor_copy(KT[:, ts(kc4, 512)], ps[:, 0:512])

                # V projection for this k-tile: [k(128), ev-pack(128)]
                psv = spsum.tile([P, 1024], f32, tag="sco")
                for dc in range(NDC):
                    nc.tensor.matmul(
                        psv[:, 0:P],
                        lhsT=vt_s[:, dc, ts(kt, P)],
                        rhs=wv_s[:, dc, :],
                        start=(dc == 0),
                        stop=(dc == NDC - 1),
                    )
                nc.vector.tensor_copy(Vf[:, kt, :], psv[:, 0:P])

                for h in range(2):
                    hp = h * DH  # partition offset of this head's QT/KT rows
                    E = epool.tile([P, L], bf16, tag="E")
                    for half in range(2):
                        ps = spsum.tile([P, 1024], f32, tag="sco")
                        for j in range(2):
                            qc = half * 2 + j
                            nc.tensor.matmul(
                                ps[:, ts(j, 512)],
                                lhsT=KT[hp : hp + DH, ts(kt, P)],
                                rhs=QT[hp : hp + DH, ts(qc, 512)],
                                start=True,
                                stop=True,
                            )
                        nc.scalar.activation(
                            E[:, ts(half, 1024)],
                            ps[:],
                            AF.Exp,
                            scale=SCALE,
                            accum_out=Dsum[:, h : h + 1, kt : kt + 1, half : half + 1],
                        )
                    # D for these 128 k-rows is complete
                    nc.vector.tensor_add(
                        Dtot[:, h : h + 1, kt : kt + 1],
                        Dsum[:, h : h + 1, kt : kt + 1, 0:1],
                        Dsum[:, h : h + 1, kt : kt + 1, 1:2],
                    )
                    nc.vector.reciprocal(
                        Drec[:, h : h + 1, kt : kt + 1],
                        Dtot[:, h : h + 1, kt : kt + 1],
                    )
                    # V~ = V / D, zero-padded into the other head's half
                    Vs = vspool.tile([P, P], bf16, tag="vs")
                    nc.vector.memset(Vs[:, ts(1 - h, DH)], 0.0)
                    nc.vector.tensor_scalar_mul(
                        Vs[:, ts(h, DH)],
                        Vf[:, kt, ts(h, DH)],
                        Drec[:, h : h + 1, kt : kt + 1],
                    )
                    # out^T accumulation: rows hp:hp+64 get this head's result
                    for qc in range(NQC):
                        nc.tensor.matmul(
                            ovs[:, ts(qc, 512)],
                            lhsT=Vs[:],
                            rhs=E[:, ts(qc, 512)],
                            start=(kt == 0 and h == 0),
                            stop=(kt == NKT - 1 and h == 1),
                            skip_group_check=True,
                        )

            out_s = outp.tile([P, L], f32)
            nc.vector.tensor_copy(out_s[:], ovs[:])
            nc.sync.dma_start(out_d[:], out_s[:])

    nc.compile()
    return nc


def _get_program():
    if "nc" not in _CACHE:
        _CACHE["nc"] = _build_program()
    return _CACHE["nc"]


def kernel(keys, queries, values, WQ, WK, WV):
    import ml_dtypes

    from concourse import bass_utils

    bf = ml_dtypes.bfloat16
    keys = np.asarray(keys)
    queries = np.asarray(queries)
    values = np.asarray(values)
    WQ = np.asarray(WQ)
    WK = np.asarray(WK)
    WV = np.asarray(WV)

    nc = _get_program()

    in_maps = []
    for c in range(N_CORES):
        b = c // 4
        h0 = 2 * (c % 4)
        h1 = h0 + 1
        in_maps.append(
            {
                "qt": np.ascontiguousarray(queries[b].T).astype(bf),
                "kt": np.ascontiguousarray(keys[b].T).astype(bf),
                "vt": np.ascontiguousarray(values[b].T).astype(bf),
                "wq": np.concatenate([WQ[h0], WQ[h1]], axis=1).astype(bf),
                "wk": np.concatenate([WK[h0], WK[h1]], axis=1).astype(bf),
                "wv": np.concatenate([WV[h0], WV[h1]], axis=1).astype(bf),
            }
        )

    res = bass_utils.run_bass_kernel_spmd(nc, in_maps, core_ids=list(range(N_CORES)))

    out = np.empty((B, L, H * DH), dtype=np.float32)
    for c in range(N_CORES):
        b = c // 4
        h0 = 2 * (c % 4)
        ot = res.results[c]["out"]  # [128, L]
        out[b, :, h0 * DH : (h0 + 1) * DH] = ot[0:DH, :].T
        out[b, :, (h0 + 1) * DH : (h0 + 2) * DH] = ot[DH : 2 * DH, :].T
    return out
